# revision 40
# baseline (speedup 1.0000x reference)
"""CQAttention (BiDAF context-query attention) forward kernel for 8 Trainium2
NeuronCores — bf16 edition.

Full inputs: context (64,128,1024) f32, question (64,128,128) f32, w (384,) f32.
Full output: (64, 512, 1024) f32.

Sharding: pure data parallel over batch — 8 batches per core, w replicated.
The 2e-2 relative-error gate leaves ample room for bf16 (host emulation of the
full bf16 pipeline measures ~1.0e-3), which halves DMA bytes — the roofline
resource — and doubles DVE throughput on 16-bit ops.

Per batch (X = context[b] (H,C) bf16, Y = question[b] (H,Q) bf16):
    Z    = wcq*Y + wc                  (H,Q)
    S'_c = X_c^T @ Z   (8 chunks)      (C,Q)  -> P' = exp(S') bf16 (SBUF direct)
    tt   = sum_c P'_c-contract XT_c    (Q,H+1): XT carries a host-injected ones
           column, so tt[:,128] accumulates d = colsum(P') — the softmax
           denominators come out of the tt matmul for free.
    P    = P'^T  via 8 PE transposes (bf16 PSUM -> 2x-mode DVE/ACT copies)
    A    = (r*Y^T)^T @ P               (H,C)  = a^T
    Bm   = (r^2*tt)^T @ P              (H,C)  = b^T
    out  = [A; X*A; X*Bm]  (3H,C) bf16; block 0 (= context) is assembled
           host-side as a pure input passthrough.

X^T and Y^T are supplied by the host in an SBUF-tiled layout
(xt[b,p,c,h] = X[h,128c+p]) so their DMAs are plain contiguous 2KB-per-
partition transfers — the on-chip alternatives (DMA xbar transpose: 387B
packets; PE transposes: PSUM round-trips on the busiest engines) both lose.
"""

import os
import sys

import numpy as np

if "/opt/trn_rl_repo" not in sys.path:
    sys.path.insert(0, "/opt/trn_rl_repo")

B, H, C, Q = 64, 128, 1024, 128
NCORES = 8
BPC = B // NCORES  # batches per core
XTW = 132  # X^T chunk width: 128 data + ones col + pad
# packed input layout (per batch, per partition): [X | XT(8 chunks) | Y | YT]
OFF_XT = C
OFF_Y = C + 8 * XTW
OFF_YT = OFF_Y + Q
IN_W = OFF_YT + H


def _ensure_ntff_hook():
    """This container's `antenv` stub lacks `axon_hooks`, which
    bass_utils needs for NTFF profiling under axon (trace=True). Install
    a functional shadow module + register the ctypes-based hook."""
    import types

    try:
        from antenv.axon_hooks import get_axon_ntff_profile_hook  # noqa: F401

        return  # real module present
    except ImportError:
        pass
    try:
        import antenv

        mod = types.ModuleType("antenv.axon_hooks")
        _state = {"hook": None}

        def set_axon_ntff_profile_hook(h):
            _state["hook"] = h

        def get_axon_ntff_profile_hook():
            return _state["hook"]

        mod.set_axon_ntff_profile_hook = set_axon_ntff_profile_hook
        mod.get_axon_ntff_profile_hook = get_axon_ntff_profile_hook
        sys.modules["antenv.axon_hooks"] = mod
        antenv.axon_hooks = mod

        from trn_agent_boot.trn_boot import _ntff_profile_via_ctypes

        set_axon_ntff_profile_hook(
            _ntff_profile_via_ctypes("/opt/axon/libaxon_pjrt.so")
        )
    except Exception:
        pass  # profiling degrades; compute still works


_ensure_ntff_hook()

LAST_RESULTS = None
_NC = None


def _build():
    from contextlib import ExitStack

    import concourse.bacc as bacc
    import concourse.mybir as mybir
    import concourse.tile as tile
    from concourse import masks

    f32 = mybir.dt.float32
    f32r = mybir.dt.float32r
    bf16 = mybir.dt.bfloat16
    EXP = mybir.ActivationFunctionType.Exp

    nc = bacc.Bacc(
        "TRN2", target_bir_lowering=False, debug=False, enable_asserts=False
    )
    in_t = nc.dram_tensor("inall", (BPC, 128, IN_W), bf16, kind="ExternalInput").ap()
    w_t = nc.dram_tensor("w", (3 * H,), f32, kind="ExternalInput").ap()
    out_t = nc.dram_tensor("out", (BPC, 3 * H, C), bf16, kind="ExternalOutput").ap()

    with tile.TileContext(nc) as tc, ExitStack() as ctx:
        const = ctx.enter_context(tc.tile_pool(name="const", bufs=1))
        sb = ctx.enter_context(tc.tile_pool(name="sb", bufs=6))
        sbx = ctx.enter_context(tc.tile_pool(name="sbx", bufs=6))
        ps = ctx.enter_context(tc.tile_pool(name="ps", bufs=5, space="PSUM"))
        psb = ctx.enter_context(tc.tile_pool(name="psb", bufs=2, space="PSUM"))
        pstt = ctx.enter_context(tc.tile_pool(name="pstt", bufs=1, space="PSUM"))

        ident = const.tile([128, 128], f32, tag="ident")
        masks.make_identity(nc, ident[:])
        identr = const.tile([128, 128], f32r, tag="identr")
        nc.vector.tensor_copy(identr[:], ident[:])
        identb = const.tile([128, 128], bf16, tag="identb")
        nc.vector.tensor_copy(identb[:], ident[:])

        # w arrives as one contiguous (1,384) row; the (128,1) columns are
        # produced by K=1 PE matmuls against identity.
        w_row = const.tile([1, 3 * H], f32r, tag="w_row")
        nc.sync.dma_start(w_row[:], w_t.unsqueeze(0).bitcast(f32r))
        wc = const.tile([128, 1], f32, tag="wc")
        wcq = const.tile([128, 1], f32, tag="wcq")

        state = {}  # keyed by batch index -> dict of live tiles

        def stage1(b):
            # one packed input DMA per batch: 4.6KB/partition contiguous
            st = {}
            IN = sbx.tile([128, IN_W], bf16, tag="IN")
            # first few prefetches go via the (idle-at-start) SWDGE queue so
            # the pipeline fill isn't serialized on one queue; steady-state
            # inputs stay on sync where waits can't block compute
            (nc.gpsimd if b < 3 else nc.sync).dma_start(IN[:], in_t[b])
            X = IN[:, 0:C]
            Y = IN[:, OFF_Y : OFF_Y + Q]
            YT = IN[:, OFF_YT : OFF_YT + H]

            if b == 0:
                wps = ps.tile([128, 512], f32, tag="s512")
                nc.tensor.matmul(
                    wps[:, 0:128],
                    w_row[0:1, H : 2 * H],
                    identr[0:1, 0:128],
                    start=True,
                    stop=True,
                )
                nc.tensor.matmul(
                    wps[:, 128:256],
                    w_row[0:1, 2 * H : 3 * H],
                    identr[0:1, 0:128],
                    start=True,
                    stop=True,
                )
                nc.vector.tensor_copy(wc[:], wps[:, 0:1])
                nc.vector.tensor_copy(wcq[:], wps[:, 128:129])

            # Z = wcq * Y + wc on Pool (SBUF-only; Pool is otherwise idle)
            Z = sb.tile([H, Q], bf16, tag="Z")
            nc.gpsimd.tensor_scalar(
                Z[:],
                Y[:],
                wcq[:],
                wc[:],
                mybir.AluOpType.mult,
                mybir.AluOpType.add,
            )
            st.update(IN=IN, X=X, YT=YT, Z=Z)
            return st

        def sprime_mms(b):
            # S' chunks (C,Q layout) on PE — first thing each iteration so
            # the exp' -> tt chain starts ASAP
            st = state[b]
            X, Z = st["X"], st["Z"]
            PT = sb.tile([128, C], bf16, tag="PT")
            Sps = []
            for g in range(2):
                Sp = ps.tile([128, 512], f32, tag="s512")
                for k in range(4):
                    c0 = g * 4 + k
                    nc.tensor.matmul(
                        Sp[:, k * 128 : (k + 1) * 128],
                        X[:, c0 * 128 : (c0 + 1) * 128],
                        Z[:],
                        start=True,
                        stop=True,
                    )
                Sps.append(Sp)
            st.update(PT=PT, Sps=Sps)

        def exp_g(b, g):
            st = state[b]
            nc.scalar.activation(
                st["PT"][:, g * 512 : (g + 1) * 512], st["Sps"][g][:], EXP
            )

        def exp_both(b):
            exp_g(b, 0)
            exp_g(b, 1)

        def ab_mms(b):
            # old batch's A/B matmuls + PSUM consumers: all inputs ready,
            # so these go early in every engine queue
            st = state[b]
            X, P, YTs, tts = st["X"], st["P"], st["YTs"], st["tts"]
            OUT = sb.tile([H, 3 * C], bf16, tag="OUT")
            Apss, Bpss = [], []
            for j in range(2):
                Aps = ps.tile([H, 512], f32, tag="s512")
                nc.tensor.matmul(
                    Aps[:],
                    YTs[:],
                    P[:, j * 512 : (j + 1) * 512],
                    start=True,
                    stop=True,
                )
                Apss.append(Aps)
            for j in range(2):
                nc.scalar.copy(OUT[:, j * 512 : (j + 1) * 512], Apss[j][:])
            for j in range(2):
                Bps = ps.tile([H, 512], f32, tag="s512")
                nc.tensor.matmul(
                    Bps[:],
                    tts[:],
                    P[:, j * 512 : (j + 1) * 512],
                    start=True,
                    stop=True,
                )
                Bpss.append(Bps)
            st.update(OUT=OUT, Bpss=Bpss)

        def muls_out(b):
            st = state[b]
            X, OUT, Bpss = st["X"], st["OUT"], st["Bpss"]
            last = b >= BPC - 2
            if last:
                # drain-tail batches: ship each piece the moment it's ready,
                # split across both free queues
                nc.gpsimd.dma_start(out_t[b, 0:H], OUT[:, 0:C])  # A block
                for j in range(2):
                    nc.vector.tensor_mul(
                        OUT[:, 2 * C + j * 512 : 2 * C + (j + 1) * 512],
                        X[:, j * 512 : (j + 1) * 512],
                        Bpss[j][:],
                    )
                    nc.sync.dma_start(
                        out_t[b, 2 * H : 3 * H, j * 512 : (j + 1) * 512],
                        OUT[:, 2 * C + j * 512 : 2 * C + (j + 1) * 512],
                    )
                nc.vector.tensor_mul(OUT[:, C : 2 * C], X[:], OUT[:, 0:C])
                nc.gpsimd.dma_start(out_t[b, H : 2 * H], OUT[:, C : 2 * C])
            else:
                for j in range(2):
                    # X*B straight from PSUM (B itself is never output)
                    nc.vector.tensor_mul(
                        OUT[:, 2 * C + j * 512 : 2 * C + (j + 1) * 512],
                        X[:, j * 512 : (j + 1) * 512],
                        Bpss[j][:],
                    )
                # X*A all-bf16 (2x DVE mode), one wide op
                nc.vector.tensor_mul(OUT[:, C : 2 * C], X[:], OUT[:, 0:C])
                nc.gpsimd.dma_start(out_t[b, 2 * H : 3 * H], OUT[:, 2 * C : 3 * C])
                nc.gpsimd.dma_start(
                    out_t[b, 0 : 2 * H].rearrange("(blk r) c -> r blk c", blk=2),
                    OUT[:, 0 : 2 * C],
                )

        def tt_ptr(b):
            st = state[b]
            IN, YT, PT = st["IN"], st["YT"], st["PT"]
            # tt = P @ X^T (Q,H); col 128 accumulates d = colsum(P') via the
            # host-injected ones column in XT
            tt = pstt.tile([Q, XTW], f32, tag="tt")
            for c in range(8):
                nc.tensor.matmul(
                    tt[:],
                    PT[:, c * 128 : (c + 1) * 128],
                    IN[:, OFF_XT + c * XTW : OFF_XT + (c + 1) * XTW],
                    start=(c == 0),
                    stop=(c == 7),
                )
            # P = P'^T via PE transposes (bf16 PSUM), 2x-mode copies
            P = sb.tile([Q, C], bf16, tag="P")
            for g in range(2):
                Pp = psb.tile([128, 512], bf16, tag="ptp")
                for k in range(4):
                    c0 = g * 4 + k
                    nc.tensor.transpose(
                        Pp[:, k * 128 : (k + 1) * 128],
                        PT[:, c0 * 128 : (c0 + 1) * 128],
                        identb[:],
                    )
                if g == 0:
                    nc.vector.tensor_copy(P[:, 0:512], Pp[:])
                else:
                    nc.scalar.copy(P[:, 512:1024], Pp[:])
            # softmax denominators out of tt's ones column
            rr = sb.tile([Q, 1], f32, tag="rr")
            nc.vector.reciprocal(rr[:], tt[:, 128:129])
            r2 = sb.tile([Q, 1], f32, tag="r2")
            nc.vector.tensor_mul(r2[:], rr[:], rr[:])
            YTs = sb.tile([Q, H], bf16, tag="YTs")
            nc.vector.tensor_scalar_mul(YTs[:], YT[:], rr[:])
            tts = sb.tile([Q, H], bf16, tag="tts")
            nc.vector.tensor_scalar_mul(tts[:], tt[:, 0:128], r2[:])
            st.update(P=P, YTs=YTs, tts=tts)

        # 4-deep software pipeline; at iteration start every emitted op's
        # inputs come from previous iterations, so each engine queue is
        # immediately executable:
        #   it: DMA(b) | S'+exp'(b-1) | A/B+copies+muls+out(b-3) | tt/Ptr(b-2)
        for it in range(BPC + 3):
            b1, b2, b3, b4 = it, it - 1, it - 2, it - 3
            if b1 < BPC:
                state[b1] = stage1(b1)
            if 0 <= b2 < BPC:
                sprime_mms(b2)
                exp_both(b2)
            if 0 <= b4:
                ab_mms(b4)
            if 0 <= b3 < BPC:
                tt_ptr(b3)
            if 0 <= b4:
                muls_out(b4)
                del state[b4]

    nc.compile()
    return nc


def kernel(context, question, w):
    global _NC, LAST_RESULTS
    import ml_dtypes
    from concourse import bass_utils

    if _NC is None:
        _NC = _build()

    bf16 = ml_dtypes.bfloat16
    context = np.asarray(context)
    question = np.asarray(question)
    ctx16 = np.ascontiguousarray(context.astype(bf16))
    q16 = np.ascontiguousarray(question.astype(bf16))
    w = np.ascontiguousarray(np.asarray(w), dtype=np.float32)

    # packed per-batch input: [X | XT tiled (xt[b,p,c,h]=X[b,h,128c+p], ones
    # col at 128) | Y | YT], one contiguous 4.6KB/partition DMA
    inall = np.zeros((B, 128, IN_W), dtype=bf16)
    inall[:, :, 0:C] = ctx16
    xt = inall[:, :, OFF_XT : OFF_XT + 8 * XTW].reshape(B, 128, 8, XTW)
    xt[..., 0:128] = (
        ctx16.transpose(0, 2, 1).reshape(B, 8, 128, H).transpose(0, 2, 1, 3)
    )
    xt[..., 128] = np.asarray(1.0, dtype=bf16)
    inall[:, :, OFF_Y : OFF_Y + Q] = q16
    inall[:, :, OFF_YT : OFF_YT + H] = q16.transpose(0, 2, 1)

    in_maps = [
        {
            "inall": inall[c * BPC : (c + 1) * BPC],
            "w": w,
        }
        for c in range(NCORES)
    ]
    trace = bool(int(os.environ.get("KTRACE", "0")))
    LAST_RESULTS = bass_utils.run_bass_kernel_spmd(
        _NC, in_maps, core_ids=list(range(NCORES)), trace=trace
    )
    out = np.empty((B, 4 * H, C), dtype=np.float32)
    out[:, 0:H, :] = np.asarray(context, dtype=np.float32)
    for c in range(NCORES):
        out[c * BPC : (c + 1) * BPC, H:, :] = LAST_RESULTS.results[c][
            "out"
        ].astype(np.float32)
    return out


# revision 45
# speedup vs baseline: 1.2298x; 1.2298x over previous
"""CQAttention (BiDAF context-query attention) forward kernel for 8 Trainium2
NeuronCores — bf16 edition.

Full inputs: context (64,128,1024) f32, question (64,128,128) f32, w (384,) f32.
Full output: (64, 512, 1024) f32.

Sharding: pure data parallel over batch — 8 batches per core, w replicated.
The 2e-2 relative-error gate leaves ample room for bf16 (host emulation of the
full bf16 pipeline measures ~1.0e-3), which halves DMA bytes — the roofline
resource — and doubles DVE throughput on 16-bit ops.

Per batch (X = context[b] (H,C) bf16, Y = question[b] (H,Q) bf16):
    Z    = wcq*Y + wc                  (H,Q)
    S'_c = X_c^T @ Z   (8 chunks)      (C,Q)  -> P' = exp(S') bf16 (SBUF direct)
    tt   = sum_c P'_c-contract XT_c    (Q,H+1): XT carries a host-injected ones
           column, so tt[:,128] accumulates d = colsum(P') — the softmax
           denominators come out of the tt matmul for free.
    P    = P'^T  via 8 PE transposes (bf16 PSUM -> 2x-mode DVE/ACT copies)
    A    = (r*Y^T)^T @ P               (H,C)  = a^T
    Bm   = (r^2*tt)^T @ P              (H,C)  = b^T
    out  = [A; X*A; X*Bm]  (3H,C) bf16; block 0 (= context) is assembled
           host-side as a pure input passthrough.

X^T and Y^T are supplied by the host in an SBUF-tiled layout
(xt[b,p,c,h] = X[h,128c+p]) so their DMAs are plain contiguous 2KB-per-
partition transfers — the on-chip alternatives (DMA xbar transpose: 387B
packets; PE transposes: PSUM round-trips on the busiest engines) both lose.
"""

import os
import sys

import numpy as np

if "/opt/trn_rl_repo" not in sys.path:
    sys.path.insert(0, "/opt/trn_rl_repo")

B, H, C, Q = 64, 128, 1024, 128
NCORES = 8
BPC = B // NCORES  # batches per core
XTW = 132  # X^T chunk width: 128 data + ones col + pad
# packed input layout (per batch, per partition): [X | XT(8 chunks) | Y | YT]
OFF_XT = C
OFF_Y = C + 8 * XTW
OFF_YT = OFF_Y + Q
IN_W = OFF_YT + H


def _ensure_ntff_hook():
    """This container's `antenv` stub lacks `axon_hooks`, which
    bass_utils needs for NTFF profiling under axon (trace=True). Install
    a functional shadow module + register the ctypes-based hook."""
    import types

    try:
        from antenv.axon_hooks import get_axon_ntff_profile_hook  # noqa: F401

        return  # real module present
    except ImportError:
        pass
    try:
        import antenv

        mod = types.ModuleType("antenv.axon_hooks")
        _state = {"hook": None}

        def set_axon_ntff_profile_hook(h):
            _state["hook"] = h

        def get_axon_ntff_profile_hook():
            return _state["hook"]

        mod.set_axon_ntff_profile_hook = set_axon_ntff_profile_hook
        mod.get_axon_ntff_profile_hook = get_axon_ntff_profile_hook
        sys.modules["antenv.axon_hooks"] = mod
        antenv.axon_hooks = mod

        from trn_agent_boot.trn_boot import _ntff_profile_via_ctypes

        set_axon_ntff_profile_hook(
            _ntff_profile_via_ctypes("/opt/axon/libaxon_pjrt.so")
        )
    except Exception:
        pass  # profiling degrades; compute still works


_ensure_ntff_hook()

LAST_RESULTS = None
_NC = None


def _build():
    from contextlib import ExitStack

    import concourse.bacc as bacc
    import concourse.mybir as mybir
    import concourse.tile as tile
    from concourse import masks

    f32 = mybir.dt.float32
    f32r = mybir.dt.float32r
    bf16 = mybir.dt.bfloat16
    EXP = mybir.ActivationFunctionType.Exp

    nc = bacc.Bacc(
        "TRN2", target_bir_lowering=False, debug=False, enable_asserts=False
    )
    in_t = nc.dram_tensor("inall", (BPC, 128, IN_W), bf16, kind="ExternalInput").ap()
    w_t = nc.dram_tensor("w", (3 * H,), f32, kind="ExternalInput").ap()
    out_t = nc.dram_tensor("out", (BPC, 3 * H, C), bf16, kind="ExternalOutput").ap()

    with tile.TileContext(nc) as tc, ExitStack() as ctx:
        const = ctx.enter_context(tc.tile_pool(name="const", bufs=1))
        sb = ctx.enter_context(tc.tile_pool(name="sb", bufs=6))
        sbx = ctx.enter_context(tc.tile_pool(name="sbx", bufs=9))
        ps = ctx.enter_context(tc.tile_pool(name="ps", bufs=5, space="PSUM"))
        psb = ctx.enter_context(tc.tile_pool(name="psb", bufs=2, space="PSUM"))
        pstt = ctx.enter_context(tc.tile_pool(name="pstt", bufs=1, space="PSUM"))

        ident = const.tile([128, 128], f32, tag="ident")
        masks.make_identity(nc, ident[:])
        identr = const.tile([128, 128], f32r, tag="identr")
        nc.vector.tensor_copy(identr[:], ident[:])
        identb = const.tile([128, 128], bf16, tag="identb")
        nc.vector.tensor_copy(identb[:], ident[:])

        # w arrives as one contiguous (1,384) row; the (128,1) columns are
        # produced by K=1 PE matmuls against identity.
        w_row = const.tile([1, 3 * H], f32r, tag="w_row")
        nc.sync.dma_start(w_row[:], w_t.unsqueeze(0).bitcast(f32r))
        wc = const.tile([128, 1], f32, tag="wc")
        wcq = const.tile([128, 1], f32, tag="wcq")

        state = {}  # keyed by batch index -> dict of live tiles

        def stage0(b, eng):
            # one packed input DMA per batch: 4.6KB/partition contiguous
            st = {}
            IN = sbx.tile([128, IN_W], bf16, tag="IN")
            eng.dma_start(IN[:], in_t[b])
            st.update(
                IN=IN,
                X=IN[:, 0:C],
                Y=IN[:, OFF_Y : OFF_Y + Q],
                YT=IN[:, OFF_YT : OFF_YT + H],
            )
            state[b] = st

        def stage1(b):
            st = state[b]
            Y = st["Y"]

            if b == 0:
                wps = ps.tile([128, 512], f32, tag="s512")
                nc.tensor.matmul(
                    wps[:, 0:128],
                    w_row[0:1, H : 2 * H],
                    identr[0:1, 0:128],
                    start=True,
                    stop=True,
                )
                nc.tensor.matmul(
                    wps[:, 128:256],
                    w_row[0:1, 2 * H : 3 * H],
                    identr[0:1, 0:128],
                    start=True,
                    stop=True,
                )
                nc.vector.tensor_copy(wc[:], wps[:, 0:1])
                nc.vector.tensor_copy(wcq[:], wps[:, 128:129])

            # Z = wcq * Y + wc on Pool (SBUF-only; Pool is otherwise idle)
            Z = sb.tile([H, Q], bf16, tag="Z")
            nc.gpsimd.tensor_scalar(
                Z[:],
                Y[:],
                wcq[:],
                wc[:],
                mybir.AluOpType.mult,
                mybir.AluOpType.add,
            )
            st.update(Z=Z)

        def sprime_mms(b):
            # S' chunks (C,Q layout) on PE — first thing each iteration so
            # the exp' -> tt chain starts ASAP
            st = state[b]
            X, Z = st["X"], st["Z"]
            PT = sb.tile([128, C], bf16, tag="PT")
            Sps = []
            for g in range(2):
                Sp = ps.tile([128, 512], f32, tag="s512")
                for k in range(4):
                    c0 = g * 4 + k
                    nc.tensor.matmul(
                        Sp[:, k * 128 : (k + 1) * 128],
                        X[:, c0 * 128 : (c0 + 1) * 128],
                        Z[:],
                        start=True,
                        stop=True,
                    )
                Sps.append(Sp)
            st.update(PT=PT, Sps=Sps)

        def exp_g(b, g):
            st = state[b]
            nc.scalar.activation(
                st["PT"][:, g * 512 : (g + 1) * 512], st["Sps"][g][:], EXP
            )

        def exp_both(b):
            exp_g(b, 0)
            exp_g(b, 1)

        def ab_mms(b):
            # old batch's A/B matmuls + PSUM consumers: all inputs ready,
            # so these go early in every engine queue
            st = state[b]
            X, P, YTs, tts = st["X"], st["P"], st["YTs"], st["tts"]
            OUT = sb.tile([H, 3 * C], bf16, tag="OUT")
            Apss, Bpss = [], []
            for j in range(2):
                Aps = ps.tile([H, 512], f32, tag="s512")
                nc.tensor.matmul(
                    Aps[:],
                    YTs[:],
                    P[:, j * 512 : (j + 1) * 512],
                    start=True,
                    stop=True,
                )
                Apss.append(Aps)
            for j in range(2):
                nc.scalar.copy(OUT[:, j * 512 : (j + 1) * 512], Apss[j][:])
            for j in range(2):
                Bps = ps.tile([H, 512], f32, tag="s512")
                nc.tensor.matmul(
                    Bps[:],
                    tts[:],
                    P[:, j * 512 : (j + 1) * 512],
                    start=True,
                    stop=True,
                )
                Bpss.append(Bps)
            st.update(OUT=OUT, Bpss=Bpss)

        def muls_out(b):
            st = state[b]
            X, OUT, Bpss = st["X"], st["OUT"], st["Bpss"]
            for j in range(2):
                # X*B straight from PSUM (B itself is never output)
                nc.vector.tensor_mul(
                    OUT[:, 2 * C + j * 512 : 2 * C + (j + 1) * 512],
                    X[:, j * 512 : (j + 1) * 512],
                    Bpss[j][:],
                )
            # X*A all-bf16 (2x DVE mode), one wide op
            nc.vector.tensor_mul(OUT[:, C : 2 * C], X[:], OUT[:, 0:C])
            # XB block; last batches drain on the sync queue in parallel
            eng = nc.sync if b >= BPC - 2 else nc.gpsimd
            eng.dma_start(out_t[b, 2 * H : 3 * H], OUT[:, 2 * C : 3 * C])
            nc.gpsimd.dma_start(
                out_t[b, 0 : 2 * H].rearrange("(blk r) c -> r blk c", blk=2),
                OUT[:, 0 : 2 * C],
            )

        def tt_ptr(b):
            st = state[b]
            IN, YT, PT = st["IN"], st["YT"], st["PT"]
            # tt = P @ X^T (Q,H); col 128 accumulates d = colsum(P') via the
            # host-injected ones column in XT
            tt = pstt.tile([Q, XTW], f32, tag="tt")
            for c in range(8):
                nc.tensor.matmul(
                    tt[:],
                    PT[:, c * 128 : (c + 1) * 128],
                    IN[:, OFF_XT + c * XTW : OFF_XT + (c + 1) * XTW],
                    start=(c == 0),
                    stop=(c == 7),
                )
            # P = P'^T via PE transposes (bf16 PSUM), 2x-mode copies
            P = sb.tile([Q, C], bf16, tag="P")
            for g in range(2):
                Pp = psb.tile([128, 512], bf16, tag="ptp")
                for k in range(4):
                    c0 = g * 4 + k
                    nc.tensor.transpose(
                        Pp[:, k * 128 : (k + 1) * 128],
                        PT[:, c0 * 128 : (c0 + 1) * 128],
                        identb[:],
                    )
                if g == 0:
                    nc.vector.tensor_copy(P[:, 0:512], Pp[:])
                else:
                    nc.scalar.copy(P[:, 512:1024], Pp[:])
            # softmax denominators out of tt's ones column
            rr = sb.tile([Q, 1], f32, tag="rr")
            nc.vector.reciprocal(rr[:], tt[:, 128:129])
            r2 = sb.tile([Q, 1], f32, tag="r2")
            nc.vector.tensor_mul(r2[:], rr[:], rr[:])
            YTs = sb.tile([Q, H], bf16, tag="YTs")
            nc.vector.tensor_scalar_mul(YTs[:], YT[:], rr[:])
            tts = sb.tile([Q, H], bf16, tag="tts")
            nc.vector.tensor_scalar_mul(tts[:], tt[:, 0:128], r2[:])
            st.update(P=P, YTs=YTs, tts=tts)

        # Prefetch the first inputs back-to-back, alternating the two free
        # DMA queues, BEFORE any dependent op can block either queue —
        # arrivals then come in pairs instead of serializing on one queue.
        NPRE = min(4, BPC)
        for b in range(NPRE):
            stage0(b, nc.gpsimd if b % 2 else nc.sync)

        # 4-deep software pipeline; at iteration start every emitted op's
        # inputs come from previous iterations, so each engine queue is
        # immediately executable:
        #   it: DMA(b) | S'+exp'(b-1) | A/B+copies+muls+out(b-3) | tt/Ptr(b-2)
        for it in range(BPC + 3):
            b0, b1, b2, b3, b4 = it + NPRE, it, it - 1, it - 2, it - 3
            if b0 < BPC:
                stage0(b0, nc.sync)
            if b1 < BPC:
                stage1(b1)
            if 0 <= b2 < BPC:
                sprime_mms(b2)
                exp_both(b2)
            if 0 <= b4:
                ab_mms(b4)
            if 0 <= b3 < BPC:
                tt_ptr(b3)
            if 0 <= b4:
                muls_out(b4)
                del state[b4]

    nc.compile()
    return nc


def kernel(context, question, w):
    global _NC, LAST_RESULTS
    import ml_dtypes
    from concourse import bass_utils

    if _NC is None:
        _NC = _build()

    bf16 = ml_dtypes.bfloat16
    context = np.asarray(context)
    question = np.asarray(question)
    ctx16 = np.ascontiguousarray(context.astype(bf16))
    q16 = np.ascontiguousarray(question.astype(bf16))
    w = np.ascontiguousarray(np.asarray(w), dtype=np.float32)

    # packed per-batch input: [X | XT tiled (xt[b,p,c,h]=X[b,h,128c+p], ones
    # col at 128) | Y | YT], one contiguous 4.6KB/partition DMA
    inall = np.zeros((B, 128, IN_W), dtype=bf16)
    inall[:, :, 0:C] = ctx16
    xt = inall[:, :, OFF_XT : OFF_XT + 8 * XTW].reshape(B, 128, 8, XTW)
    xt[..., 0:128] = (
        ctx16.transpose(0, 2, 1).reshape(B, 8, 128, H).transpose(0, 2, 1, 3)
    )
    xt[..., 128] = np.asarray(1.0, dtype=bf16)
    inall[:, :, OFF_Y : OFF_Y + Q] = q16
    inall[:, :, OFF_YT : OFF_YT + H] = q16.transpose(0, 2, 1)

    in_maps = [
        {
            "inall": inall[c * BPC : (c + 1) * BPC],
            "w": w,
        }
        for c in range(NCORES)
    ]
    trace = bool(int(os.environ.get("KTRACE", "0")))
    LAST_RESULTS = bass_utils.run_bass_kernel_spmd(
        _NC, in_maps, core_ids=list(range(NCORES)), trace=trace
    )
    out = np.empty((B, 4 * H, C), dtype=np.float32)
    out[:, 0:H, :] = np.asarray(context, dtype=np.float32)
    for c in range(NCORES):
        out[c * BPC : (c + 1) * BPC, H:, :] = LAST_RESULTS.results[c][
            "out"
        ].astype(np.float32)
    return out
